# revision 52
# baseline (speedup 1.0000x reference)
"""MemEffEquivariantAttention TRN2 Bass kernel (transposed-scores flow, v5).

Sharding: 8 cores = 4 batches x 2 query-token halves (fully data-parallel,
no collectives).

Scores are computed TRANSPOSED (wT[s, t] = kT_chunk.T @ qT) so the
attention probabilities already have s on partitions and feed the attn
matmul directly.  Z (softmax denominator) is accumulated per group of 4
heads into one PSUM tile via one-hot E4 lhsT matmuls; normalization is
applied LATE on the stashed X tile (v4 scheme).

v5 changes (driven by the 120us v4 trace):
  - queue discipline: the Scalar (ACT) queue issues ZERO DMAs -- in v4
    the first exp sat behind a vS dma_start waiting for ring credits,
    stalling PE+DVE+ACT 14us at startup.  All input loads are per-head
    so head h's data lands just-in-time; vS is re-laid-out per head.
  - o_sb scaling folded into ACT activation (scale=inv per-partition).

v6 changes (driven by the 151us v5 trace: 5-10us PE stalls on m0/u;
xnorm pieces sat in front of m0/u in the in-order DVE queue waiting on
stash DMAs that queued behind vS loads on the gpsimd ring):
  - k/q merged host-side into one [H, D, S+TQ] tensor.
  - m0/u back to per-hf ops (fused [128,2048] STT/TT lengthened the
    exp->m0->u->attn chain; per-hf overlaps exp(hf1) with m0(hf0)).

v14/v15 (final, ~116-118us vs 119.9us baseline).  Key measured facts:
one issuing ring's hardware DMA queue tops out ~250GB/s; two queues
~300GB/s aggregate (per-core share with all 8 cores loading).  Loads
alternate WHOLE tensors between the sync and gpsimd rings by head
parity (keeps 4KB descriptors); eb+vS are fused into one ebv tensor
(fewer ring-credit cycles); the rz reciprocal (fast-approx) is split
from its rzX broadcast DMA so the broadcast's semaphore wait never
head-of-line-blocks a load ring; 6-head prefetch depth (bufs=8)
decouples the load stream from compute hiccups; X-stash p-slices and
per-p output stores are spread across both rings.
"""
import sys
sys.path.insert(0, "/opt/trn_rl_repo")

import numpy as np
import ml_dtypes

import concourse.bacc as bacc
import concourse.tile as tile
from concourse import mybir
from concourse.bass_utils import run_bass_kernel_spmd

F32 = mybir.dt.float32
F16 = mybir.dt.float16
BF16 = mybir.dt.bfloat16
AF = mybir.ActivationFunctionType
ALU = mybir.AluOpType

B, T, P, HID = 4, 512, 3, 512
HD, H = 32, 16
EXP, S = 512, 1024
TQ = 256            # query tokens per core
EPS = 1e-3
CUTOFF = 1e-5
NEG = -1e30
D = P * HD          # 96, per-head feature dim
SHIFT = -40.0       # constant softmax shift, cancels exactly via Z

USE_STT = False     # scalar_tensor_tensor measured SLOWER than tensor_tensor

_prog_cache = {}


def _build_program():
    nc = bacc.Bacc("TRN2", target_bir_lowering=False, debug=False)

    # kq[h, :, :S] = kT, kq[h, :, S:] = qT (merged: one DMA per head)
    kq_d = nc.dram_tensor("kq", [H, D, S + TQ], F16, kind="ExternalInput").ap()
    # vS[part, h, sc, d] = vf[sc*128+part, h-th head, d]
    # ebv[h] = concat(eb[h] flattened, vS[h] flattened) along free dim:
    # eb = exp(masked bias)^T [128, 2, 1024]; vS [128, 8, 96] -- one DMA
    EBW = 2 * 4 * TQ          # 2048 eb words per partition
    VW = 8 * D                # 768 vS words per partition
    ebv_d = nc.dram_tensor("ebv", [H, 128, EBW + VW], BF16,
                           kind="ExternalInput").ap()
    lawT_d = nc.dram_tensor("lawT", [128, 2, 4 * TQ], BF16, kind="ExternalInput").ap()
    WT_d = nc.dram_tensor("WT", [128, 4, HID], BF16, kind="ExternalInput").ap()
    E4_d = nc.dram_tensor("E4", [128, 4, 4], BF16, kind="ExternalInput").ap()
    ones128f_d = nc.dram_tensor("ones128f", [128, 1], F32, kind="ExternalInput").ap()
    out_d = nc.dram_tensor("out", [TQ, P, HID], BF16, kind="ExternalOutput").ap()

    def vmul(out, a, b):
        if USE_STT:
            nc.vector.scalar_tensor_tensor(out, a, 1.0, b, ALU.mult, ALU.mult)
        else:
            nc.vector.tensor_mul(out, a, b)

    with tile.TileContext(nc) as tc:
        with tc.tile_pool(name="const", bufs=1) as cp, \
             tc.tile_pool(name="kq", bufs=8) as kq, \
             tc.tile_pool(name="ebp", bufs=8) as ebp, \
             tc.tile_pool(name="vp", bufs=8) as vp, \
             tc.tile_pool(name="eu", bufs=4) as eu, \
             tc.tile_pool(name="work", bufs=3) as wp, \
             tc.tile_pool(name="psw", bufs=2, space="PSUM") as psw, \
             tc.tile_pool(name="psa", bufs=2, space="PSUM") as psa, \
             tc.tile_pool(name="psz", bufs=2, space="PSUM") as psz:

            # ---- constants ----
            lawT_t = cp.tile([128, 2, 4 * TQ], BF16, tag="lawT")
            WT_t = cp.tile([128, 4, HID], BF16, tag="WT")
            E4_t = cp.tile([128, 4, 4], BF16, tag="E4")
            ones128f_t = cp.tile([128, 1], F32, tag="o128f")
            eps_t = cp.tile([128, 1], F32, tag="eps")
            shift_t = cp.tile([128, 1], F32, tag="shift")
            X_t = cp.tile([128, P, 4, TQ], BF16, tag="X")
            rzX_t = cp.tile([128, 4, TQ], F32, tag="rzX")
            sqacc_t = cp.tile([128, TQ], F32, tag="sqacc")
            nc.vector.memset(eps_t[:], EPS)
            nc.vector.memset(shift_t[:], SHIFT)

            kq_tiles, ebv_tiles = {}, {}
            m0_tiles, u_tiles, z4_tiles, at_tiles = {}, {}, {}, {}

            def eb_ap(h):
                return ebv_tiles[h][:, :EBW].rearrange(
                    "p (f x) -> p f x", f=2)

            def vS_ap(h):
                return ebv_tiles[h][:, EBW:].rearrange(
                    "p (s d) -> p s d", s=8)

            def emit_load_head(h):
                # split streams across BOTH hardware DMA queues: a single
                # queue tops out at ~250GB/s, the aggregate at ~390GB/s.
                # eb+vS are fused into one 5.5KB/partition DMA.
                kq_tiles[h] = kq.tile([D, S + TQ], F16, tag="kq",
                                      name=f"kq_{h}")
                ebv_tiles[h] = ebp.tile([128, EBW + VW], BF16, tag="ebv",
                                        name=f"ebv_{h}")
                ring_a = nc.sync if h % 2 == 0 else nc.gpsimd
                ring_b = nc.gpsimd if h % 2 == 0 else nc.sync
                ring_a.dma_start(out=ebv_tiles[h][:], in_=ebv_d[h])
                ring_b.dma_start(out=kq_tiles[h][:], in_=kq_d[h])

            def emit_scores(h):
                kq_t = kq_tiles[h]
                e_t = eu.tile([128, 2, 4 * TQ], BF16, tag="e", name=f"e_{h}")
                m0_t = eu.tile([128, 2, 4 * TQ], BF16, tag="m0",
                               name=f"m0_{h}")
                u_t = eu.tile([128, 2, 4 * TQ], BF16, tag="u", name=f"u_{h}")
                for hf in range(2):
                    w_ps = psw.tile([128, 4 * TQ], F32, tag="w",
                                    name=f"w_{h}_{hf}")
                    for sc4 in range(4):
                        sc = 4 * hf + sc4
                        nc.tensor.matmul(w_ps[:, sc4 * TQ:(sc4 + 1) * TQ],
                                         kq_t[:, sc * 128:(sc + 1) * 128],
                                         kq_t[:, S:],
                                         start=True, stop=True,
                                         skip_group_check=True)
                    nc.scalar.activation(e_t[:, hf, :], w_ps[:], AF.Exp,
                                         bias=shift_t[:])
                    vmul(m0_t[:, hf, :], e_t[:, hf, :],
                         eb_ap(h)[:, hf, :])
                    vmul(u_t[:, hf, :], m0_t[:, hf, :], lawT_t[:, hf, :])
                m0_tiles[h], u_tiles[h] = m0_t, u_t

            def emit_post_pe(h):
                g, h4 = h // 4, h % 4
                m0_t, u_t = m0_tiles[h], u_tiles[h]
                if h4 == 0:
                    z4_tiles[g] = psz.tile([4, TQ], F32, tag="z",
                                           name=f"z4_{g}")
                z4_ps = z4_tiles[g]
                at_ps = psa.tile([D, TQ], F32, tag="at", name=f"at_{h}")
                # interleave z/attn by hf so sc 0-3 start as soon as the
                # hf0 slice of m0/u is ready.
                # z: one-hot lhsT, head h4 writes row h4, zeros elsewhere,
                # so the whole group accumulates into one [4, 256] tile
                for hf in range(2):
                    for sc4 in range(4):
                        sc = 4 * hf + sc4
                        nc.tensor.matmul(z4_ps[:], E4_t[:, h4, :],
                                         m0_t[:, hf,
                                              sc4 * TQ:(sc4 + 1) * TQ],
                                         start=(h4 == 0 and sc == 0),
                                         stop=(h4 == 3 and sc == 7),
                                         skip_group_check=True)
                    for sc4 in range(4):
                        sc = 4 * hf + sc4
                        nc.tensor.matmul(at_ps[:], vS_ap(h)[:, sc, :],
                                         u_t[:, hf,
                                             sc4 * TQ:(sc4 + 1) * TQ],
                                         start=(sc == 0), stop=(sc == 7))
                at_tiles[h] = at_ps

            def emit_post_rest(h):
                del m0_tiles[h], u_tiles[h]
                del kq_tiles[h], ebv_tiles[h]
                at_ps = at_tiles.pop(h)
                at_sb = wp.tile([D, TQ], BF16, tag="atsb", name=f"atsb_{h}")
                nc.scalar.activation(at_sb[:], at_ps[:], AF.Copy)
                # stash into X[(h%4)*32+j, p, h//4, t] for out_proj lhsT
                # (p-slices split across rings to balance issue cost; for
                # the final heads lean on sync so the tail xnorm isn't
                # gated by one queue's serial stash latency)
                rings = ([nc.gpsimd, nc.sync, nc.sync] if h >= H - 2 else
                         [nc.gpsimd, nc.gpsimd, nc.sync])
                for p in range(P):
                    rings[p].dma_start(
                        out=X_t[(h % 4) * 32:(h % 4 + 1) * 32, p, h // 4, :],
                        in_=at_sb[p * 32:(p + 1) * 32, :])

            rz4_tiles = {}

            def emit_rz_recip(g):
                # rz for the 4 heads of group g, in one fast reciprocal
                # (Z is well-scaled: no 0/inf/denorm)
                rz4_t = wp.tile([4, TQ], F32, tag="rz4", name=f"rz4_{g}")
                nc.vector.reciprocal_approx_fast(rz4_t[:],
                                                 z4_tiles.pop(g)[:])
                rz4_tiles[g] = rz4_t

            def emit_rz_bcast(g, ring=None):
                # replicate each rz4 row 32x across partitions via one
                # stride-0-source DMA: rzX[hm*32+j, g, :] = rz4[hm, :].
                # Emitted one head AFTER the reciprocal so this DMA's wait
                # never head-of-line-blocks the load ring.
                (ring or nc.gpsimd).dma_start(
                    out=rzX_t[:, g, :],
                    in_=rz4_tiles.pop(g)[:].unsqueeze(1)
                        .broadcast_to([4, 32, TQ]))

            def emit_xnorm_piece(g, p, rows=128):
                # normalize one p-slice of X in place (DVE; feeds out_proj)
                # + its sumsq piece; spread one piece per head so no queue
                # sees a burst.  rows<128 skips head-rows that were
                # pre-normalized before their stash (last head's fast path).
                r = slice(0, rows)
                vmul(X_t[r, p, g, :], X_t[r, p, g, :], rzX_t[r, g, :])
                if g == 0 and p == 0:
                    vmul(sqacc_t[:], X_t[:, p, g, :], X_t[:, p, g, :])
                else:
                    sq_t = wp.tile([128, TQ], BF16, tag="sq")
                    vmul(sq_t[r], X_t[r, p, g, :], X_t[r, p, g, :])
                    nc.vector.tensor_add(sqacc_t[r], sqacc_t[r], sq_t[r])

            # ---- prologue loads: head 0/1 payloads lead both rings ----
            emit_load_head(0)
            nc.sync.dma_start(out=lawT_t[:], in_=lawT_d)
            emit_load_head(1)
            nc.gpsimd.dma_start(out=E4_t[:], in_=E4_d)
            emit_load_head(2)
            emit_load_head(3)
            emit_load_head(4)
            emit_load_head(5)
            nc.gpsimd.dma_start(out=ones128f_t[:], in_=ones128f_d)
            nc.gpsimd.dma_start(out=WT_t[:], in_=WT_d)

            # ---- main loop, software-pipelined by one head ----
            pieces = []
            for h in range(H):
                if h + 6 < H:
                    emit_load_head(h + 6)
                emit_scores(h)
                if h >= 1:
                    emit_post_pe(h - 1)
                    emit_post_rest(h - 1)
                if h % 4 == 0 and h >= 4:
                    # group h//4-1's z4 got its stop in emit_post_pe(h-1)
                    emit_rz_recip(h // 4 - 1)
                    pieces += [(h // 4 - 1, p) for p in range(P)]
                else:
                    if h % 4 == 1 and h >= 5:
                        emit_rz_bcast(h // 4 - 1)
                    if pieces:
                        emit_xnorm_piece(*pieces.pop(0))
            emit_post_pe(H - 1)
            emit_rz_recip(3)
            # fast path for the last head: broadcast its rz row to the
            # 96 at-partitions, normalize + square BEFORE the stash, so
            # the out_proj depends only on the stash DMA and the group-3
            # xnorm pieces (heads 12-14, stashed long ago) run in
            # parallel with it instead of after it.
            rzA_t = wp.tile([96, TQ], F32, tag="rzA")
            nc.sync.dma_start(
                out=rzA_t[:],
                in_=rz4_tiles[3][3:4, :].unsqueeze(1)
                    .broadcast_to([1, 96, TQ]))
            hL = H - 1
            del m0_tiles[hL], u_tiles[hL], kq_tiles[hL], ebv_tiles[hL]
            at_ps = at_tiles.pop(hL)
            at_sb = wp.tile([D, TQ], BF16, tag="atsb", name=f"atsb_{hL}")
            nc.scalar.activation(at_sb[:], at_ps[:], AF.Copy)
            vmul(at_sb[:], at_sb[:], rzA_t[:])
            sq96_t = wp.tile([96, TQ], F32, tag="sq96")
            vmul(sq96_t[:], at_sb[:], at_sb[:])
            rings = [nc.gpsimd, nc.sync, nc.gpsimd]
            for p in range(P):
                rings[p].dma_start(out=X_t[96:128, p, 3, :],
                                   in_=at_sb[p * 32:(p + 1) * 32, :])
            emit_rz_bcast(3, ring=nc.sync)
            for gp in pieces:
                emit_xnorm_piece(*gp)
            for p in range(P):
                emit_xnorm_piece(3, p, rows=96)

            # ---- inv = 1/sqrt(mean+eps), out_proj, scale, store ----
            ss_ps = [psz.tile([128, 1], F32, tag="z", name=f"ss{tb}")
                     for tb in range(2)]
            for tb in range(2):
                nc.tensor.matmul(ss_ps[tb][:],
                                 sqacc_t[:, tb * 128:(tb + 1) * 128],
                                 ones128f_t[:], start=True, stop=False)
                # head 15's squares live in (p,hd)-partition layout;
                # the partition sum doesn't care about the layout
                nc.tensor.matmul(ss_ps[tb][:],
                                 sq96_t[:, tb * 128:(tb + 1) * 128],
                                 ones128f_t[:96, :], start=False, stop=True)
            inv_t = []
            for tb in range(2):
                tmp_t = wp.tile([128, 1], F32, tag=f"tmp{tb}")
                nc.scalar.activation(tmp_t[:], ss_ps[tb][:], AF.Sqrt,
                                     scale=1.0 / HID, bias=eps_t[:])
                iv = wp.tile([128, 1], F32, tag=f"inv{tb}")
                nc.vector.reciprocal(iv[:], tmp_t[:])
                inv_t.append(iv)

            for tb in range(2):
                o_sb = wp.tile([128, P, HID], BF16, tag="osb",
                               name=f"osb_{tb}")
                for p in range(P):
                    o_ps = psw.tile([128, HID], F32, tag="w",
                                    name=f"o_{p}_{tb}")
                    for ci in range(4):
                        nc.tensor.matmul(o_ps[:],
                                         X_t[:, p, ci, tb * 128:(tb + 1) * 128],
                                         WT_t[:, ci, :],
                                         start=(ci == 0), stop=(ci == 3))
                    # scale by inv (per-partition = per-t) on ACT, then
                    # store each p-slice immediately (alternating rings)
                    nc.scalar.activation(o_sb[:, p, :], o_ps[:], AF.Copy,
                                         scale=inv_t[tb][:])
                    ring = nc.gpsimd if (tb * P + p) % 2 == 0 else nc.sync
                    ring.dma_start(out=out_d[tb * 128:(tb + 1) * 128, p, :],
                                   in_=o_sb[:, p, :])

    nc.compile()
    return nc


def _get_program():
    if "nc" not in _prog_cache:
        _prog_cache["nc"] = _build_program()
    return _prog_cache["nc"]


def _prepare_in_maps(q, k, v, attn_bias, key_padding_mask, outcell_index,
                     local_attention_weight, expand_mask, out_proj_weight,
                     attn_ln_weight):
    q = np.asarray(q, dtype=np.float32)
    k = np.asarray(k, dtype=np.float32)
    v = np.asarray(v, dtype=np.float32)
    attn_bias = np.asarray(attn_bias, dtype=np.float32)
    kpm = np.asarray(key_padding_mask)
    idx = np.asarray(outcell_index).astype(np.int64)
    law = np.asarray(local_attention_weight, dtype=np.float32)
    emask = np.asarray(expand_mask)
    W = np.asarray(out_proj_weight, dtype=np.float32)
    lnw = np.asarray(attn_ln_weight, dtype=np.float32)

    WT = np.ascontiguousarray((W * lnw[None, :]).T)  # [hid, o], ln folded
    E4_np = np.zeros((128, 4, 4), dtype=ml_dtypes.bfloat16)
    for i in range(4):
        E4_np[:, i, i] = 1
    ones128f_np = np.ones((128, 1), dtype=np.float32)

    in_maps = []
    for c in range(8):
        b, th = c // 2, c % 2
        tsl = slice(th * TQ, (th + 1) * TQ)

        # kT [H, 96, S]: kf[s, p, h*32+hd] with s-expansion host-gathered
        kf = np.concatenate([k[b], k[b][idx[b]]], axis=0)  # [S, P, HID]
        kT = kf.reshape(S, P, H, HD).transpose(2, 1, 3, 0).reshape(H, D, S)
        qT = q[b, tsl].reshape(TQ, P, H, HD).transpose(2, 1, 3, 0) \
            .reshape(H, D, TQ)
        kqT = np.concatenate([kT, qT], axis=2)  # [H, D, S+TQ]

        # vS [128, H, 8, 96]: vS[part, h, sc, (p,hd)] = vf[sc*128+part, ...]
        vf = np.concatenate([v[b], v[b][idx[b]]], axis=0)  # [S, P, HID]
        vS = vf.reshape(8, 128, P, H, HD).transpose(1, 3, 0, 2, 4) \
            .reshape(128, H, 8 * D).transpose(1, 0, 2)  # [H, 128, 768]

        # masked bias [H, 256, S]
        bias_c = np.ascontiguousarray(attn_bias[b, :, tsl, :])
        kpmS = np.concatenate([kpm[b], emask[b]])           # [S]
        if kpmS.any():
            bias_c[:, :, kpmS] = NEG
        cut = law[b, tsl] <= CUTOFF                         # [256, S]
        if cut.any():
            bias_c[:, cut] = NEG
        # exp, transpose to [H, S, 256] -> [H, 128, 2, 1024]
        ebT = np.exp(bias_c.transpose(0, 2, 1)).reshape(H, 8, 128, TQ) \
            .transpose(0, 2, 1, 3).reshape(H, 128, 2, 4 * TQ)

        lawT = law[b, tsl].T.reshape(8, 128, TQ).transpose(1, 0, 2) \
            .reshape(128, 2, 4 * TQ)

        ebv = np.concatenate(
            [ebT.reshape(H, 128, 2 * 4 * TQ), vS], axis=2)  # [H,128,2816]

        in_maps.append(dict(
            kq=np.ascontiguousarray(kqT).astype(np.float16),
            ebv=np.ascontiguousarray(ebv).astype(ml_dtypes.bfloat16),
            lawT=np.ascontiguousarray(lawT).astype(ml_dtypes.bfloat16),
            WT=WT.reshape(4, 128, HID).transpose(1, 0, 2).astype(
                ml_dtypes.bfloat16).copy(),
            E4=E4_np,
            ones128f=ones128f_np,
        ))
    return in_maps


def kernel(**inputs):
    in_maps = _prepare_in_maps(**inputs)
    nc = _get_program()
    res = run_bass_kernel_spmd(nc, in_maps, list(range(8)))

    out = np.empty((B, T, P, HID), dtype=np.float32)
    for c in range(8):
        b, th = c // 2, c % 2
        out[b, th * TQ:(th + 1) * TQ] = res.results[c]["out"].astype(np.float32)
    return out
